# revision 1
# baseline (speedup 1.0000x reference)
"""Trainium2 Bass kernel: CANE FeatureEmbedding GNN message passing.

Strategy (node-range sharding, 8 cores):
  - Nodes are range-partitioned: core r owns nodes [r*6250, (r+1)*6250).
  - Edges are assigned to the core owning their DESTINATION (row = edge_index[1]).
  - Algebraic collapse of the peer branch (gather and scatter both use `row`):
        h_peer[i] = relu( sqrt(deg_i)*(W_px x_i + b_p) + deg_i^-1/2 * (W_pe S_ea[i]) )
    where S_ea[i] = segment_sum(edge_attrs, row). This removes the [E,160]x[160,96]
    per-edge MLP entirely.
  - Per-edge work that remains: h_e = relu(ea @ W_edge.T + b_edge), segment sums of
    ea and h_e over row, and M[i] = sum_{e: row=i} h_e_agg[col[e]].
  - Segment sums use a fixed "slot grid": each node gets C slots; edge k of node v
    goes to (block b = v//128, partition v%128, slot k). One matmul per slot tile
    accumulates S_ea directly in PSUM; h_e slots land in per-slot PSUM columns and
    are reduced after a single big ReLU. Nodes with deg > C spill to per-block
    overflow tiles handled with one-hot scatter matmuls.
  - One AllGather shares h_e_agg; M is built from an indirect-DMA gather of
    h_e_agg[col] in the same slot layout, then reduced along slots.
"""

import numpy as np

import concourse.bass as bass
import concourse.mybir as mybir
import concourse.tile as tile
from concourse import bacc
from concourse._compat import get_trn_type
from concourse.bass import IndirectOffsetOnAxis
from concourse.bass_utils import run_bass_kernel_spmd

F32 = mybir.dt.float32
I32 = mybir.dt.int32
AX = mybir.AxisListType
OP = mybir.AluOpType
ACT = mybir.ActivationFunctionType


class Cfg:
    def __init__(self, N=50000, E=800000, ncores=8, C=15, t_ov=3):
        self.N = N
        self.E = E
        self.ncores = ncores
        self.NPC = N // ncores            # nodes per core
        self.NBLK = (self.NPC + 127) // 128
        self.PADN = self.NBLK * 128       # padded nodes per core
        self.C = C                        # grid slots per node
        self.t_ov = t_ov                  # overflow tiles per block
        self.TPB = C + t_ov               # tiles per block
        self.ND = 128                     # node feature dim
        self.ED = 32                      # edge feature dim
        self.OUTD = 224

    def key(self):
        return (self.N, self.E, self.ncores, self.C, self.t_ov)


def build_program(cfg, skip=()):
    """Build the SPMD Bass program (same NEFF for all cores).

    skip: iterable of {"gather", "slotmm", "overflow", "phasec", "collective"} —
    timing-experiment knobs only (results are wrong when used)."""
    skip = set(skip)
    nc = bacc.Bacc(get_trn_type() or "TRN2", target_bir_lowering=False, debug=True)
    NBLK, TPB, C, t_ov, PADN = cfg.NBLK, cfg.TPB, cfg.C, cfg.t_ov, cfg.PADN
    TOT = NBLK * TPB

    eat = nc.declare_dram_parameter("eat", [33, TOT * 128], F32, isOutput=False)
    gidx = nc.declare_dram_parameter("gidx", [128, NBLK * TPB], I32, isOutput=False)
    rlov = nc.declare_dram_parameter("rlov", [128, NBLK * t_ov], F32, isOutput=False)
    xt = nc.declare_dram_parameter("xt", [128, PADN], F32, isOutput=False)
    dinvp = nc.declare_dram_parameter("dinv", [128, NBLK], F32, isOutput=False)
    sqdp = nc.declare_dram_parameter("sqd", [128, NBLK], F32, isOutput=False)
    sdrow = nc.declare_dram_parameter("sdrow", [1, PADN], F32, isOutput=False)
    rconst = nc.declare_dram_parameter("rconst", [33, 64], F32, isOutput=False)
    wegot = nc.declare_dram_parameter("wegot", [128, 64], F32, isOutput=False)
    wpxt = nc.declare_dram_parameter("wpxt", [128, 96], F32, isOutput=False)
    wpet = nc.declare_dram_parameter("wpet", [32, 96], F32, isOutput=False)
    bego = nc.declare_dram_parameter("bego", [1, 64], F32, isOutput=False)
    bpeer = nc.declare_dram_parameter("bpeer", [1, 96], F32, isOutput=False)
    iota = nc.declare_dram_parameter("iota", [128, 128], F32, isOutput=False)
    ident = nc.declare_dram_parameter("ident", [128, 128], F32, isOutput=False)
    outp = nc.declare_dram_parameter("out", [PADN, cfg.OUTD], F32, isOutput=True)

    with tile.TileContext(nc) as tc:
        with (
            tc.tile_pool(name="const", bufs=1) as cp,
            tc.tile_pool(name="resident", bufs=1) as rp,
            tc.tile_pool(name="dram", bufs=1, space="DRAM") as dp,
        ):
            rc = cp.tile([33, 64], F32)
            nc.sync.dma_start(rc[:], rconst[:])
            wego_sb = cp.tile([128, 64], F32)
            nc.sync.dma_start(wego_sb[:], wegot[:])
            wpx_sb = cp.tile([128, 96], F32)
            nc.sync.dma_start(wpx_sb[:], wpxt[:])
            wpe_sb = cp.tile([32, 96], F32)
            nc.sync.dma_start(wpe_sb[:], wpet[:])
            bego_sb = cp.tile([1, 64], F32)
            nc.sync.dma_start(bego_sb[:], bego[:])
            bpeer_sb = cp.tile([1, 96], F32)
            nc.sync.dma_start(bpeer_sb[:], bpeer[:])
            iota_sb = cp.tile([128, 128], F32)
            nc.sync.dma_start(iota_sb[:], iota[:])
            ident_sb = cp.tile([128, 128], F32)
            nc.sync.dma_start(ident_sb[:], ident[:])
            ones_sb = cp.tile([1, 128], F32)
            nc.gpsimd.memset(ones_sb[:], 1.0)

            xt_sb = rp.tile([128, PADN], F32)
            nc.sync.dma_start(xt_sb[:], xt[:])
            gidx_sb = rp.tile([128, NBLK * TPB], I32)
            nc.sync.dma_start(gidx_sb[:], gidx[:])
            rlov_sb = rp.tile([128, NBLK * t_ov], F32)
            nc.sync.dma_start(rlov_sb[:], rlov[:])
            dinv_sb = rp.tile([128, NBLK], F32)
            nc.sync.dma_start(dinv_sb[:], dinvp[:])
            sqd_sb = rp.tile([128, NBLK], F32)
            nc.sync.dma_start(sqd_sb[:], sqdp[:])
            sdrow_sb = rp.tile([1, PADN], F32)
            nc.sync.dma_start(sdrow_sb[:], sdrow[:])

            sea_sb = rp.tile([128, NBLK * 32], F32)
            heagg_sb = rp.tile([128, NBLK * 32], F32)

            zloc = dp.tile([PADN, 32], F32)
            zag = dp.tile([cfg.ncores * PADN, 32], F32, addr_space="Shared")

            # ---------------- Phase A: per-edge MLP + segment sums ----------
            with (
                tc.tile_pool(name="eatp", bufs=int(getattr(cfg, "eat_bufs", 3))) as eatp,
                tc.tile_pool(name="workA", bufs=int(getattr(cfg, "wa_bufs", 4))) as wp,
                tc.tile_pool(name="psA", bufs=2, space="PSUM") as psA,
                tc.tile_pool(name="psB", bufs=int(getattr(cfg, "psb_bufs", 2)), space="PSUM") as psB,
                tc.tile_pool(name="psP", bufs=int(getattr(cfg, "psp_bufs", 2)), space="PSUM") as psP,
                tc.tile_pool(name="psO", bufs=2, space="PSUM") as psO,
            ):
                for b in range(NBLK):
                    ech = eatp.tile([33, TPB * 128], F32, tag="ech")
                    nc.sync.dma_start(ech[:], eat[:, b * TPB * 128:(b + 1) * TPB * 128])
                    bankA = psA.tile([128, 32], F32, tag="bankA")
                    bankB = psB.tile([128, C * 32], F32, tag="bankB")
                    for j in range(C):
                        if "slotmm" in skip:
                            break
                        lh = ech[:, j * 128:(j + 1) * 128]
                        nc.tensor.matmul(bankA[:], lh, rc[:, 0:32],
                                         start=(j == 0), stop=(j == C - 1))
                        nc.tensor.matmul(bankB[:, j * 32:(j + 1) * 32], lh, rc[:, 32:64],
                                         start=(j == 0), stop=(j == C - 1))
                    bankO = psO.tile([128, 64], F32, tag="bankO")
                    for o in ([] if "overflow" in skip else range(t_ov)):
                        lh = ech[:, (C + o) * 128:(C + o + 1) * 128]
                        pc = psP.tile([128, 64], F32, tag="pc")
                        nc.tensor.matmul(pc[:], lh, rc[:, 0:64], start=True, stop=True)
                        ov = wp.tile([128, 64], F32, tag="ov")
                        nc.vector.tensor_copy(ov[:, 0:32], pc[:, 0:32])
                        nc.vector.tensor_scalar_max(ov[:, 32:64], pc[:, 32:64], 0.0)
                        oh = wp.tile([128, 128], F32, tag="oh")
                        k = b * t_ov + o
                        nc.vector.tensor_scalar(
                            out=oh[:], in0=iota_sb[:],
                            scalar1=rlov_sb[:, k:k + 1], scalar2=None,
                            op0=OP.is_equal,
                        )
                        nc.tensor.matmul(bankO[:], oh[:], ov[:],
                                         start=(o == 0), stop=(o == t_ov - 1))
                    relu_st = wp.tile([128, C * 32], F32, tag="relu")
                    nc.scalar.activation(relu_st[:], bankB[:], ACT.Relu)
                    t_he = wp.tile([128, 32], F32, tag="the")
                    nc.vector.tensor_reduce(
                        t_he[:],
                        relu_st[:].rearrange("p (j c) -> p c j", j=C),
                        axis=AX.X, op=OP.add,
                    )
                    ovsb = wp.tile([128, 64], F32, tag="ovsb")
                    nc.vector.tensor_copy(ovsb[:], bankO[:])
                    nc.vector.tensor_tensor(
                        out=heagg_sb[:, b * 32:(b + 1) * 32],
                        in0=t_he[:], in1=ovsb[:, 32:64], op=OP.add)
                    nc.vector.tensor_tensor(
                        out=sea_sb[:, b * 32:(b + 1) * 32],
                        in0=ovsb[:, 0:32], in1=bankA[:], op=OP.add)

            # h_e_agg -> DRAM -> AllGather
            nc.sync.dma_start(
                zloc[:].rearrange("(b p) c -> p b c", p=128),
                heagg_sb[:].rearrange("p (b c) -> p b c", c=32),
            )
            if "collective" not in skip:
                for _rep in range(int(getattr(cfg, "ag_rep", 1))):
                    nc.gpsimd.collective_compute(
                        "AllGather", OP.bypass,
                        ins=[zloc.opt()], outs=[zag.opt()],
                        replica_groups=[list(range(cfg.ncores))],
                    )

            # ------------- Phase B+C: gather/M + node-level MLPs ------------
            with (
                tc.tile_pool(name="workB", bufs=int(getattr(cfg, "wb_bufs", 4))) as wb,
                tc.tile_pool(name="outp_pool", bufs=int(getattr(cfg, "op_bufs", 3))) as op_pool,
                tc.tile_pool(name="psM", bufs=2, space="PSUM") as psM,
                tc.tile_pool(name="ps1", bufs=2, space="PSUM") as ps1,
                tc.tile_pool(name="ps2", bufs=2, space="PSUM") as ps2,
                tc.tile_pool(name="ps3", bufs=1, space="PSUM") as ps3,
                tc.tile_pool(name="psT", bufs=1, space="PSUM") as psT,
            ):
                for b in range(NBLK):
                    g = wb.tile([128, TPB * 32], F32, tag="g", bufs=8)
                    for t in ([] if "gather" in skip else range(TPB)):
                        for _rep in range(int(getattr(cfg, "gather_rep", 1))):
                            nc.gpsimd.indirect_dma_start(
                                out=g[:, t * 32:(t + 1) * 32],
                                out_offset=None,
                                in_=zag[:],
                                in_offset=IndirectOffsetOnAxis(
                                    ap=gidx_sb[:, b * TPB + t:b * TPB + t + 1], axis=0),
                            )
                    m_main = wb.tile([128, 32], F32, tag="mmain")
                    nc.vector.tensor_reduce(
                        m_main[:],
                        g[:, 0:C * 32].rearrange("p (j c) -> p c j", j=C),
                        axis=AX.X, op=OP.add,
                    )
                    pm = psM.tile([128, 32], F32, tag="pm")
                    for o in range(t_ov):
                        oh = wb.tile([128, 128], F32, tag="oh2")
                        k = b * t_ov + o
                        nc.vector.tensor_scalar(
                            out=oh[:], in0=iota_sb[:],
                            scalar1=rlov_sb[:, k:k + 1], scalar2=None,
                            op0=OP.is_equal,
                        )
                        nc.tensor.matmul(pm[:], oh[:], g[:, (C + o) * 32:(C + o + 1) * 32],
                                         start=(o == 0), stop=(o == t_ov - 1))
                    outst = op_pool.tile([128, cfg.OUTD], F32, tag="outst")
                    t_m = wb.tile([128, 32], F32, tag="tm")
                    nc.vector.tensor_tensor(out=t_m[:], in0=m_main[:], in1=pm[:], op=OP.add)
                    nc.vector.tensor_scalar_mul(outst[:, 96:128], t_m[:], dinv_sb[:, b:b + 1])
                    nc.vector.tensor_copy(outst[:, 64:96], heagg_sb[:, b * 32:(b + 1) * 32])

                    # h_ego = relu(x W_ego^T + b_ego)
                    if "phasec" in skip:
                        nc.sync.dma_start(outp[b * 128:(b + 1) * 128, :], outst[:])
                        continue
                    p1 = ps1.tile([128, 64], F32, tag="p1")
                    nc.tensor.matmul(p1[:], ones_sb[:], bego_sb[:], start=True, stop=False)
                    nc.tensor.matmul(p1[:], xt_sb[:, b * 128:(b + 1) * 128], wego_sb[:],
                                     start=False, stop=True)
                    nc.vector.tensor_scalar_max(outst[:, 0:64], p1[:], 0.0)

                    # h_peer = relu(sqd*(W_px x) + sqd*b_p + W_pe (dinv*S_ea))
                    p2 = ps2.tile([128, 96], F32, tag="p2")
                    nc.tensor.matmul(p2[:], xt_sb[:, b * 128:(b + 1) * 128], wpx_sb[:],
                                     start=True, stop=True)
                    p3 = ps3.tile([128, 96], F32, tag="p3")
                    nc.tensor.matmul(p3[:], sdrow_sb[:, b * 128:(b + 1) * 128], bpeer_sb[:],
                                     start=True, stop=False)
                    t_s = wb.tile([128, 32], F32, tag="ts")
                    nc.vector.tensor_scalar_mul(t_s[:], sea_sb[:, b * 32:(b + 1) * 32],
                                                dinv_sb[:, b:b + 1])
                    pt = psT.tile([32, 128], F32, tag="pt")
                    nc.tensor.matmul(pt[:], t_s[:], ident_sb[:], is_transpose=True,
                                     start=True, stop=True)
                    seat = wb.tile([32, 128], F32, tag="seat")
                    nc.vector.tensor_copy(seat[:], pt[:])
                    nc.tensor.matmul(p3[:], seat[:], wpe_sb[:], start=False, stop=True)
                    t_u = wb.tile([128, 96], F32, tag="tu")
                    nc.vector.tensor_scalar_mul(t_u[:], p2[:], sqd_sb[:, b:b + 1])
                    nc.vector.tensor_tensor(out=t_u[:], in0=t_u[:], in1=p3[:], op=OP.add)
                    nc.vector.tensor_scalar_max(outst[:, 128:224], t_u[:], 0.0)

                    nc.sync.dma_start(outp[b * 128:(b + 1) * 128, :], outst[:])
    nc.compile()
    return nc


def host_prep(cfg, x, edge_attrs, edge_index):
    """Shard + lay out inputs for the slot-grid kernel. Pure index work + O(N)
    scalar prep (degree normalizers); all O(E*H)/O(N*H) FP math runs on device."""
    N, E, C, NBLK, TPB, t_ov, NPC, PADN = (cfg.N, cfg.E, cfg.C, cfg.NBLK,
                                           cfg.TPB, cfg.t_ov, cfg.NPC, cfg.PADN)
    row = np.asarray(edge_index[1]).astype(np.int64)
    col = np.asarray(edge_index[0]).astype(np.int64)
    ea = np.asarray(edge_attrs, dtype=np.float32)
    xf = np.asarray(x, dtype=np.float32)

    deg = np.bincount(row, minlength=N)
    degf = np.maximum(deg, 1).astype(np.float64)
    dinv = np.where(deg > 0, degf ** -0.5, 0.0).astype(np.float32)
    sqd = np.sqrt(deg.astype(np.float64)).astype(np.float32)

    core = row // NPC
    lrow = row - core * NPC
    blk = lrow // 128
    part = lrow % 128

    # rank of each edge within its destination node
    order = np.argsort(row, kind="stable")
    sorted_row = row[order]
    starts = np.searchsorted(sorted_row, np.arange(N), side="left")
    rank = np.empty(E, np.int64)
    rank[order] = np.arange(E) - starts[sorted_row]

    is_grid = rank < C
    ovsel = ~is_grid
    ove = np.where(ovsel)[0]
    ovkey = core[ove] * NBLK + blk[ove]
    o_order = np.argsort(ovkey, kind="stable")
    ove = ove[o_order]
    okey_sorted = ovkey[o_order]
    ostarts = np.searchsorted(okey_sorted, np.arange(NBLK * cfg.ncores), side="left")
    opos = np.arange(ove.size) - ostarts[okey_sorted]
    otile = C + opos // 128
    opart = opos % 128
    if ove.size and otile.max() >= TPB:
        raise ValueError("overflow tiles exceeded; raise t_ov")

    # tile index + within-tile partition for every edge
    tile_idx = np.empty(E, np.int64)
    tpart = np.empty(E, np.int64)
    ge = np.where(is_grid)[0]
    tile_idx[ge] = blk[ge] * TPB + rank[ge]
    tpart[ge] = part[ge]
    tile_idx[ove] = blk[ove] * TPB + otile
    tpart[ove] = opart

    zrow = (col // NPC) * PADN + (col % NPC)     # row in allgathered z table
    assert NPC < PADN, "pad-slot gathers need a guaranteed-zero dummy row"
    ZPAD = PADN - 1                               # core0 dummy node -> zeros

    TOTC = NBLK * TPB
    in_maps = []
    # constants shared by all cores are built once
    consts = None
    for r in range(cfg.ncores):
        sel = core == r
        e_idx = np.where(sel)[0]
        t_i = tile_idx[e_idx]
        t_p = tpart[e_idx]
        colpos = t_i * 128 + t_p

        EAT = np.zeros((33, TOTC * 128), np.float32)
        EAT[:32, colpos] = ea[e_idx].T
        EAT[32, colpos] = 1.0

        GIDX = np.full((128, TOTC), ZPAD, np.int32)
        GIDX[t_p, t_i] = zrow[e_idx].astype(np.int32)

        RLOV = np.full((128, NBLK * t_ov), 200.0, np.float32)
        ovm = sel[ove] if False else None
        ov_r = ove[core[ove] == r]
        op_r = opart[core[ove] == r]
        ot_r = otile[core[ove] == r]
        ob_r = blk[ov_r]
        RLOV[op_r, ob_r * t_ov + (ot_r - C)] = part[ov_r].astype(np.float32)

        lo, hi = r * NPC, (r + 1) * NPC
        XT = np.zeros((128, PADN), np.float32)
        XT[:, :NPC] = xf[lo:hi].T
        dl = np.zeros(PADN, np.float32)
        dl[:NPC] = dinv[lo:hi]
        sl = np.zeros(PADN, np.float32)
        sl[:NPC] = sqd[lo:hi]
        DINV = dl.reshape(NBLK, 128).T.copy()
        SQD = sl.reshape(NBLK, 128).T.copy()
        SDROW = sl.reshape(1, PADN)

        m = {
            "eat": EAT, "gidx": GIDX, "rlov": RLOV, "xt": XT,
            "dinv": DINV, "sqd": SQD, "sdrow": SDROW,
        }
        in_maps.append(m)
    return in_maps


def make_consts(cfg, W_peer, b_peer, W_ego, b_ego, W_edge, b_edge):
    RCONST = np.zeros((33, 64), np.float32)
    RCONST[:32, :32] = np.eye(32, dtype=np.float32)
    RCONST[:32, 32:64] = np.asarray(W_edge, np.float32).T
    RCONST[32, 32:64] = np.asarray(b_edge, np.float32)
    consts = {
        "rconst": RCONST,
        "wegot": np.ascontiguousarray(np.asarray(W_ego, np.float32).T),
        "wpxt": np.ascontiguousarray(np.asarray(W_peer, np.float32)[:, :128].T),
        "wpet": np.ascontiguousarray(np.asarray(W_peer, np.float32)[:, 128:].T),
        "bego": np.asarray(b_ego, np.float32).reshape(1, 64),
        "bpeer": np.asarray(b_peer, np.float32).reshape(1, 96),
        "iota": np.broadcast_to(np.arange(128, dtype=np.float32), (128, 128)).copy(),
        "ident": np.eye(128, dtype=np.float32),
    }
    return consts


_CACHE = {}
RUN_KWARGS = {}


def kernel(x, edge_attrs, W_peer, b_peer, W_ego, b_ego, W_edge, b_edge, edge_index):
    x = np.asarray(x)
    edge_attrs = np.asarray(edge_attrs)
    edge_index = np.asarray(edge_index)
    N, E = x.shape[0], edge_attrs.shape[0]

    # pick t_ov from the actual degree distribution (>=3 keeps NEFF cache warm
    # for the expected data)
    row = edge_index[1].astype(np.int64)
    C = 15
    ncores = 8
    NPC = N // ncores
    NBLK = (NPC + 127) // 128
    deg = np.bincount(row, minlength=N)
    ovn = np.maximum(deg - C, 0)
    nodes = np.arange(N)
    bkey = (nodes // NPC) * NBLK + (nodes % NPC) // 128
    ovblk = np.bincount(bkey, weights=ovn.astype(np.float64), minlength=NBLK * ncores)
    t_ov = max(3, int(np.ceil(ovblk.max() / 128.0)))

    cfg = Cfg(N=N, E=E, ncores=ncores, C=C, t_ov=t_ov)
    key = cfg.key()
    if key not in _CACHE:
        _CACHE[key] = build_program(cfg)
    nc = _CACHE[key]

    in_maps = host_prep(cfg, x, edge_attrs, edge_index)
    consts = make_consts(cfg, W_peer, b_peer, W_ego, b_ego, W_edge, b_edge)
    for m in in_maps:
        m.update(consts)

    res = run_bass_kernel_spmd(nc, in_maps, core_ids=list(range(cfg.ncores)),
                               **RUN_KWARGS)
    out = np.empty((N, cfg.OUTD), np.float32)
    for r in range(cfg.ncores):
        out[r * cfg.NPC:(r + 1) * cfg.NPC] = res.results[r]["out"][:cfg.NPC]
    if RUN_KWARGS:
        kernel.last_result = res
    return out

